# revision 55
# baseline (speedup 1.0000x reference)
"""Trainium2 Bass kernel for nn_Actor (gnn_message_passing).

Math (per batch b):
  k_mu = kv[..., :128], v_mu = kv[..., 128:256]
  rel[n,m]  = <k_mu[n], v_mu[m]> / sqrt(128)
  P[n,m,:]  = pos[n] - pos[m];  Pn = P / (||P|| + eps)
  out[n,:]  = 0.01 * tanh( sum_m Pn[n,m,:] * rel[n,m] )

Factored form (avoids materializing [N,N,3]):
  W[n,m]   = rel[n,m] / ||P[n,m]||
  out[n,d] = 0.01 * tanh( pos16[n,d] * s[n] - (W @ [pos16|1])[n,d] ),
  s[n] = sum_m W[n,m].  The diagonal W[n,n] cancels exactly between the
  two terms because the same fp16 positions are used on both sides.

On-device pipeline per core (2 batches, data-parallel over B=16 / 8 cores):
  - kv loaded over three DMA queues (2 raw-fp32 halves + 1 gpsimd
    cast-DMA half); engine copies cast the fp32 halves to fp16
  - d2^T[m,n] = |p_n - p_m|^2 via one K=14 fp16 matmul: operands
    [p(3),1,1,h,l] x [-2p(3),h,l,1,1] with |p|^2 split fp16 hi+lo,
    both stacked in one [L|R] buffer; a single PE transpose pass plus an
    SBUF->SBUF DMA row-swap builds lhsT and rhs (result = 2*d2)
  - ScalarE ACT Abs_reciprocal_sqrt(E/2 * d2) -> ninv fp16 (folds the
    1/sqrt(E); |.| launders tiny-negative d2 from PSUM rounding)
  - VectorE custom op: W = rel * min(ninv, CAP) fp16 (CAP kills the
    rsqrt(~0)=inf on the diagonal)
  - deferred P[4,N] += [pos16|1]^T @ W^T burst per batch, epilogue
    transposes P to n-major, pre = pos16*s - A, single deferred tanh
  - output dumped in SBUF-native layout (contiguous DMA); host reorders
"""

import time

import numpy as np

import concourse.bass as bass
import concourse.bacc as bacc
import concourse.mybir as mybir
import concourse.tile as tile
import concourse.dve_ops as dve_ops
from concourse.bass_utils import run_bass_kernel_spmd
from concourse.dve_spec import Spec, minn
from concourse.dve_uop import DveOpSpec
from concourse.masks import make_identity

F32 = mybir.dt.float32
F16 = mybir.dt.float16
AF = mybir.ActivationFunctionType

B, N, CKV = 16, 1024, 259
E = 128
NCORES = 8
BPC = B // NCORES          # batches per core
NT = N // 128              # 128-row m-tiles per batch
KA = 7                     # aug rows per side (K=2*KA matmul computes 2*d2)
ACTION_SCALE = 0.01
CAP = float(np.float16(1.0 / np.sqrt(E * 5e-6)))   # ninv cap (d2 floor 5e-6)


def _register_capmul():
    name = "CAPMUL_GNN"
    if name in dve_ops._SUB_OPCODE_FOR_NAME:
        return next(op for op in dve_ops.OPS if op.name == name)
    from concourse.dve_spec import Src0, Src1, C0, lower

    body = Src0 * minn(Src1, C0)

    def _ref(in0, in1, s0, s1, imm2):
        in0 = np.asarray(in0, np.float32)
        in1 = np.asarray(in1, np.float32)
        return (in0 * np.minimum(in1, np.float32(s0))).astype(np.float32)

    spec = Spec(body=body, reference=_ref)
    opcode = dve_ops._CUSTOM_DVE_ROW_BASE + len(dve_ops.OPS)
    shas = {}
    for ver in ("v3", "v4"):
        try:
            uops = lower(spec, ver=ver)
            shas[ver] = DveOpSpec(
                name=name, opcode=opcode, uops=uops, rd1_en=True
            ).sha(ver)
        except Exception:
            pass
    op = dve_ops.DveOp(name, spec, subdim=False, uops_sha=shas)
    dve_ops.OPS.append(op)
    dve_ops.CUSTOM_DVE_SPECS[name] = spec
    dve_ops._SUB_OPCODE_FOR_NAME[name] = opcode
    return op


CAPMUL_GNN = _register_capmul()


def build_nc():
    nc = bacc.Bacc("TRN2", target_bir_lowering=False, debug=False)
    kv_ext = nc.declare_dram_parameter("kv", [BPC, N, CKV], F32, isOutput=False)
    pos_ext = nc.declare_dram_parameter("positions", [BPC, N, 3], F32, isOutput=False)
    # SBUF-native layout [p, b, t, d] (row n = 8p + t); host rearranges
    out_ext = nc.declare_dram_parameter("out", [128, BPC, NT, 3], F32, isOutput=True)

    with tile.TileContext(nc) as tc:
        with (
            tc.tile_pool(name="const", bufs=1) as constp,
            tc.tile_pool(name="kv32", bufs=1) as kv32p,
            tc.tile_pool(name="kv16", bufs=1) as kv16p,
            tc.tile_pool(name="kvT", bufs=2) as kvTp,
            tc.tile_pool(name="aug", bufs=2) as augp,
            tc.tile_pool(name="ninv", bufs=8) as ninvp,
            tc.tile_pool(name="wt", bufs=16) as wtp,
            tc.tile_pool(name="epi", bufs=2) as epip,
            tc.tile_pool(name="psd2", bufs=2, space="PSUM") as psd2,
            tc.tile_pool(name="psrel", bufs=4, space="PSUM") as psrel,
        ):
            # ---- kv loads FIRST across three DMA queues ----
            kv32s, kv16s, posf = {}, {}, {}

            def kvsrc(b, h):
                src = kv_ext[b].rearrange("(p t) c -> p t c", p=128)
                return src[:, (NT // 2) * h : (NT // 2) * (h + 1), 0 : 2 * E]

            kv16s[0] = [
                kv16p.tile([128, NT // 2, 2 * E], F16, tag=f"kv0{h}",
                           name=f"kv16_0{h}")
                for h in range(2)
            ]
            kv16s[1] = [
                kv16p.tile([128, NT // 2, 2 * E], F16, tag=f"kv1{h}",
                           name=f"kv16_1{h}")
                for h in range(2)
            ]
            kv32_00 = kv32p.tile([128, NT // 2, 2 * E], F32, tag="kv32a")
            kv32_11 = kv32p.tile([128, NT // 2, 2 * E], F32, tag="kv32b")
            nc.sync.dma_start(out=kv32_00[:, :, :], in_=kvsrc(0, 0))
            for b in range(BPC):
                posf[b] = augp.tile([128, NT, 3], F32, tag="posf", name=f"posf{b}")
                nc.scalar.dma_start(
                    out=posf[b][:, :, :],
                    in_=pos_ext[b].rearrange("(p t) d -> p t d", p=128),
                )
            nc.gpsimd.dma_start(out=kv16s[0][1][:, :, :], in_=kvsrc(0, 1))
            nc.sync.dma_start(out=kv32_11[:, :, :], in_=kvsrc(1, 1))
            nc.gpsimd.dma_start(out=kv16s[1][0][:, :, :], in_=kvsrc(1, 0))

            # ---- PE warm-up primer ----
            warm_in = constp.tile([128, 512], F16)
            nc.vector.memset(warm_in[:, :], 0.0)
            warm_ps = psrel.tile([128, 512], F32, tag="rel")
            for i in range(7):
                nc.tensor.matmul(
                    warm_ps[:, :],
                    lhsT=warm_in[:, 0:128],
                    rhs=warm_in[:, :],
                    start=(i == 0),
                    stop=(i == 6),
                )
            warm_sink = constp.tile([128, 1], F32)
            nc.scalar.copy(warm_sink[:, :], warm_ps[:, 0:1])

            identity16 = constp.tile([128, 128], F16)
            make_identity(nc, identity16[:, :])
            identity32 = constp.tile([128, 128], F32)
            make_identity(nc, identity32[:, :])

            pre_all = constp.tile([128, BPC, NT, 3], F32)

            kmuT, vmuT, augTL, augTR, X, pos16f = {}, {}, {}, {}, {}, {}
            wts = {b: [] for b in range(BPC)}

            # engine casts for the raw-fp32 kv halves
            nc.vector.tensor_copy(kv16s[0][0][:, :, :], kv32_00[:, :, :])
            nc.scalar.copy(kv16s[1][1][:, :, :], kv32_11[:, :, :])

            def emit_aug(b):
                pf = posf[b]
                pos16 = augp.tile([128, NT, 3], F16, tag="p16")
                nc.gpsimd.tensor_copy(pos16[:, :, :], pf[:, :, :])
                pos16f[b] = augp.tile(
                    [128, NT, 3], F32, tag="p16f", name=f"p16f{b}"
                )
                nc.gpsimd.tensor_copy(pos16f[b][:, :, :], pos16[:, :, :])
                sq3 = augp.tile([128, NT, 3], F32, tag="sq3")
                nc.gpsimd.tensor_mul(
                    sq3[:, :, :], pos16f[b][:, :, :], pos16f[b][:, :, :]
                )
                pn2 = augp.tile([128, NT, 1], F32, tag="pn2")
                nc.vector.tensor_reduce(
                    out=pn2[:, :, :],
                    in_=sq3[:, :, :],
                    op=mybir.AluOpType.add,
                    axis=mybir.AxisListType.X,
                )
                h16 = augp.tile([128, NT, 1], F16, tag="h16")
                nc.gpsimd.tensor_copy(h16[:, :, :], pn2[:, :, :])
                h32 = augp.tile([128, NT, 1], F32, tag="h32")
                nc.gpsimd.tensor_copy(h32[:, :, :], h16[:, :, :])
                l32 = augp.tile([128, NT, 1], F32, tag="l32")
                nc.gpsimd.tensor_sub(l32[:, :, :], pn2[:, :, :], h32[:, :, :])

                # [L(7)|R(7)] in n-major; L=[p,1,1,h,l], R=[-2p,h,l,1,1]
                augb = augp.tile([128, NT, 2 * KA], F16, tag="augb")
                nc.gpsimd.tensor_copy(augb[:, :, 0:3], pos16[:, :, :])
                nc.gpsimd.memset(augb[:, :, 3:5], 1.0)
                nc.gpsimd.tensor_copy(augb[:, :, 5:6], h16[:, :, :])
                nc.gpsimd.tensor_copy(augb[:, :, 6:7], l32[:, :, :])
                nc.gpsimd.tensor_scalar_mul(augb[:, :, 7:10], pos16[:, :, :], -2.0)
                nc.gpsimd.tensor_copy(augb[:, :, 10:11], h16[:, :, :])
                nc.gpsimd.tensor_copy(augb[:, :, 11:12], l32[:, :, :])
                nc.gpsimd.memset(augb[:, :, 12:14], 1.0)

                aT_ps = psd2.tile([2 * KA, N], F16, tag="d2")
                for t in range(NT):
                    nc.tensor.transpose(
                        aT_ps[:, 128 * t : 128 * (t + 1)],
                        augb[:, t, :],
                        identity16[:, :],
                    )
                augTL[b] = augp.tile(
                    [2 * KA, N], F16, tag="augTL", name=f"augTL{b}"
                )
                # b1's drains go to scalar: the vector queue must stay free of
                # kv/pos-b1-dependent work so capmuls aren't blocked behind it
                if b == 0:
                    nc.vector.tensor_copy(augTL[b][:, :], aT_ps[:, :])
                else:
                    nc.scalar.copy(augTL[b][:, :], aT_ps[:, :])
                # row-swapped [R;L] via SBUF->SBUF DMA (engines are
                # partition-locked; DMA is not)
                augTR[b] = augp.tile(
                    [2 * KA, N], F16, tag="augTR", name=f"augTR{b}"
                )
                nc.sync.dma_start(
                    out=augTR[b][0:KA, :], in_=augTL[b][KA : 2 * KA, :]
                )
                nc.sync.dma_start(
                    out=augTR[b][KA : 2 * KA, :], in_=augTL[b][0:KA, :]
                )

                X[b] = augp.tile([128, NT, 4], F16, tag="X", name=f"X{b}")
                nc.gpsimd.tensor_copy(X[b][:, :, 0:3], pos16[:, :, :])
                nc.gpsimd.memset(X[b][:, :, 3:4], 1.0)

            def emit_kvT(b):
                kT_ps = psd2.tile([128, N], F16, tag="d2")
                for t in range(NT):
                    kv16 = kv16s[b][t // (NT // 2)]
                    tt = t % (NT // 2)
                    nc.tensor.transpose(
                        kT_ps[:, 128 * t : 128 * (t + 1)],
                        kv16[:, tt, 0:E],
                        identity16[:, :],
                    )
                kmuT[b] = kvTp.tile([128, N], F16, tag="kmuT", name=f"kmuT{b}")
                if b == 0:
                    nc.vector.tensor_copy(kmuT[b][:, :], kT_ps[:, :])
                else:
                    nc.scalar.copy(kmuT[b][:, :], kT_ps[:, :])
                vT_ps = psd2.tile([128, N], F16, tag="d2")
                for t in range(NT):
                    kv16 = kv16s[b][t // (NT // 2)]
                    tt = t % (NT // 2)
                    nc.tensor.transpose(
                        vT_ps[:, 128 * t : 128 * (t + 1)],
                        kv16[:, tt, E : 2 * E],
                        identity16[:, :],
                    )
                vmuT[b] = kvTp.tile([128, N], F16, tag="vmuT", name=f"vmuT{b}")
                nc.scalar.copy(vmuT[b][:, :], vT_ps[:, :])

            def emit_tile(b, t):
                d2_ps = psd2.tile([128, N], F32, tag="d2")
                for h in range(2):
                    cs = slice(512 * h, 512 * (h + 1))
                    nc.tensor.matmul(
                        d2_ps[:, cs],
                        lhsT=augTL[b][:, 128 * t : 128 * (t + 1)],
                        rhs=augTR[b][:, cs],
                        start=True,
                        stop=True,
                    )
                ninv = ninvp.tile([128, N], F16)
                nc.scalar.activation(
                    ninv[:, :], d2_ps[:, :], AF.Abs_reciprocal_sqrt,
                    scale=float(E) / 2.0,
                )
                w16 = wtp.tile([128, N], F16)
                for h in range(2):
                    cs = slice(512 * h, 512 * (h + 1))
                    rel_ps = psrel.tile([128, 512], F32, tag="rel")
                    nc.tensor.matmul(
                        rel_ps[:, :],
                        lhsT=vmuT[b][:, 128 * t : 128 * (t + 1)],
                        rhs=kmuT[b][:, cs],
                        start=True,
                        stop=True,
                    )
                    nc.vector._custom_dve(
                        CAPMUL_GNN,
                        out=w16[:, cs],
                        in0=rel_ps[:, :],
                        in1=ninv[:, cs],
                        s0=CAP,
                        s1=0.0,
                    )
                wts[b].append(w16)

            # ---- main: interleave both batches, b0's first tiles lead ----
            emit_aug(0)
            emit_kvT(0)
            pair_order = [(0, 0), (0, 1), (0, 2), ("aug1", None), (0, 3)]
            rest0 = [(0, t) for t in range(4, NT)]
            rest1 = [("kvT1", None)] + [(1, t) for t in range(NT)]
            while rest0 or rest1:
                if rest1:
                    pair_order.append(rest1.pop(0))
                if rest0:
                    pair_order.append(rest0.pop(0))
            for entry in pair_order:
                if entry[0] == "aug1":
                    emit_aug(1)
                elif entry[0] == "kvT1":
                    emit_kvT(1)
                else:
                    emit_tile(*entry)

            # ---- deferred P accumulation bursts + epilogues ----
            for b in range(BPC):
                P_ps = psd2.tile([4, N], F32, tag="d2", name=f"P{b}")
                for t in range(NT):
                    for h in range(2):
                        cs = slice(512 * h, 512 * (h + 1))
                        nc.tensor.matmul(
                            P_ps[:, cs],
                            lhsT=X[b][:, t, :],
                            rhs=wts[b][t][:, cs],
                            start=(t == 0),
                            stop=(t == NT - 1),
                        )
                Psb = epip.tile([4, N], F32, tag="Psb")
                nc.scalar.copy(Psb[:, :], P_ps[:, :])
                PT_ps = psrel.tile([128, NT * 4], F32, tag="rel")
                for c in range(NT):
                    nc.tensor.transpose(
                        PT_ps[:, 4 * c : 4 * (c + 1)],
                        Psb[:, 128 * c : 128 * (c + 1)],
                        identity32[0:4, 0:4],
                    )
                PT = epip.tile([128, NT, 4], F32, tag="PT")
                nc.vector.tensor_copy(
                    PT[:, :, :], PT_ps[:, :].rearrange("p (t f) -> p t f", f=4)
                )
                tmp = epip.tile([128, NT, 3], F32, tag="tmp")
                a0, a1 = bass.broadcast_tensor_aps(pos16f[b][:, :, :], PT[:, :, 3:4])
                nc.gpsimd.tensor_mul(tmp[:, :, :], a0, a1)
                nc.gpsimd.tensor_sub(pre_all[:, b, :, :], tmp[:, :, :], PT[:, :, 0:3])

            act = constp.tile([128, BPC, NT, 3], F32)
            nc.scalar.activation(act[:, :, :, :], pre_all[:, :, :, :], AF.Tanh)
            actf = constp.tile([128, BPC, NT, 3], F32)
            nc.gpsimd.tensor_scalar_mul(actf[:, :, :, :], act[:, :, :, :], ACTION_SCALE)
            nc.sync.dma_start(out=out_ext[:, :, :, :], in_=actf[:, :, :, :])

    nc.compile()
    return nc


_NC_CACHE = {}


def _get_nc():
    if "nc" not in _NC_CACHE:
        _NC_CACHE["nc"] = build_nc()
    return _NC_CACHE["nc"]


def kernel(**inputs):
    kv = np.ascontiguousarray(np.asarray(inputs["kv"], dtype=np.float32))
    pos = np.ascontiguousarray(np.asarray(inputs["positions"], dtype=np.float32))
    assert kv.shape == (B, N, CKV) and pos.shape == (B, N, 3)
    nc = _get_nc()
    in_maps = [
        {
            "kv": kv[i * BPC : (i + 1) * BPC],
            "positions": pos[i * BPC : (i + 1) * BPC],
        }
        for i in range(NCORES)
    ]
    last_err = None
    for attempt in range(3):
        try:
            res = run_bass_kernel_spmd(nc, in_maps, core_ids=list(range(NCORES)))
            break
        except Exception as e:  # transient NRT device-state races between procs
            last_err = e
            if attempt == 2:
                raise
            time.sleep(2.0 * (attempt + 1))
    outs = res.results
    # out dump is [p, b, t, d] with row n = 8p + t -> [b, n, d]
    full = [
        np.ascontiguousarray(
            outs[i]["out"].transpose(1, 0, 2, 3).reshape(BPC, N, 3)
        )
        for i in range(NCORES)
    ]
    return np.concatenate(full, axis=0)


if __name__ == "__main__":
    rng = np.random.default_rng(0)
    kv = rng.standard_normal((B, N, CKV), dtype=np.float32)
    pos = rng.standard_normal((B, N, 3), dtype=np.float32)
    out = kernel(kv=kv, positions=pos)
    print("out", out.shape, out.dtype, float(np.abs(out).max()))


# revision 56
# speedup vs baseline: 1.0068x; 1.0068x over previous
"""Trainium2 Bass kernel for nn_Actor (gnn_message_passing).

Math (per batch b):
  k_mu = kv[..., :128], v_mu = kv[..., 128:256]
  rel[n,m]  = <k_mu[n], v_mu[m]> / sqrt(128)
  P[n,m,:]  = pos[n] - pos[m];  Pn = P / (||P|| + eps)
  out[n,:]  = 0.01 * tanh( sum_m Pn[n,m,:] * rel[n,m] )

Factored form (avoids materializing [N,N,3]):
  W[n,m]   = rel[n,m] / ||P[n,m]||
  out[n,d] = 0.01 * tanh( pos16[n,d] * s[n] - (W @ [pos16|1])[n,d] ),
  s[n] = sum_m W[n,m].  The diagonal W[n,n] cancels exactly between the
  two terms because the same fp16 positions are used on both sides.

On-device pipeline per core (2 batches, data-parallel over B=16 / 8 cores):
  - kv loaded over three DMA queues (2 raw-fp32 halves + 1 gpsimd
    cast-DMA half); engine copies cast the fp32 halves to fp16
  - d2^T[m,n] = |p_n - p_m|^2 via one K=14 fp16 matmul: operands
    [p(3),1,1,h,l] x [-2p(3),h,l,1,1] with |p|^2 split fp16 hi+lo,
    both stacked in one [L|R] buffer; a single PE transpose pass plus an
    SBUF->SBUF DMA row-swap builds lhsT and rhs (result = 2*d2)
  - ScalarE ACT Abs_reciprocal_sqrt(E/2 * d2) -> ninv fp16 (folds the
    1/sqrt(E); |.| launders tiny-negative d2 from PSUM rounding)
  - VectorE custom op: W = rel * min(ninv, CAP) fp16 (CAP kills the
    rsqrt(~0)=inf on the diagonal)
  - deferred P[4,N] += [pos16|1]^T @ W^T burst per batch, epilogue
    transposes P to n-major, pre = pos16*s - A, single deferred tanh
  - output dumped in SBUF-native layout (contiguous DMA); host reorders
"""

import time

import numpy as np

import concourse.bass as bass
import concourse.bacc as bacc
import concourse.mybir as mybir
import concourse.tile as tile
import concourse.dve_ops as dve_ops
from concourse.bass_utils import run_bass_kernel_spmd
from concourse.dve_spec import Spec, minn
from concourse.dve_uop import DveOpSpec
from concourse.masks import make_identity

F32 = mybir.dt.float32
F16 = mybir.dt.float16
AF = mybir.ActivationFunctionType

B, N, CKV = 16, 1024, 259
E = 128
NCORES = 8
BPC = B // NCORES          # batches per core
NT = N // 128              # 128-row m-tiles per batch
KA = 7                     # aug rows per side (K=2*KA matmul computes 2*d2)
ACTION_SCALE = 0.01
CAP = float(np.float16(1.0 / np.sqrt(E * 5e-6)))   # ninv cap (d2 floor 5e-6)


def _register_capmul():
    name = "CAPMUL_GNN"
    if name in dve_ops._SUB_OPCODE_FOR_NAME:
        return next(op for op in dve_ops.OPS if op.name == name)
    from concourse.dve_spec import Src0, Src1, C0, lower

    body = Src0 * minn(Src1, C0)

    def _ref(in0, in1, s0, s1, imm2):
        in0 = np.asarray(in0, np.float32)
        in1 = np.asarray(in1, np.float32)
        return (in0 * np.minimum(in1, np.float32(s0))).astype(np.float32)

    spec = Spec(body=body, reference=_ref)
    opcode = dve_ops._CUSTOM_DVE_ROW_BASE + len(dve_ops.OPS)
    shas = {}
    for ver in ("v3", "v4"):
        try:
            uops = lower(spec, ver=ver)
            shas[ver] = DveOpSpec(
                name=name, opcode=opcode, uops=uops, rd1_en=True
            ).sha(ver)
        except Exception:
            pass
    op = dve_ops.DveOp(name, spec, subdim=False, uops_sha=shas)
    dve_ops.OPS.append(op)
    dve_ops.CUSTOM_DVE_SPECS[name] = spec
    dve_ops._SUB_OPCODE_FOR_NAME[name] = opcode
    return op


CAPMUL_GNN = _register_capmul()


def build_nc():
    nc = bacc.Bacc("TRN2", target_bir_lowering=False, debug=False)
    kv_ext = nc.declare_dram_parameter("kv", [BPC, N, CKV], F32, isOutput=False)
    pos_ext = nc.declare_dram_parameter("positions", [BPC, N, 3], F32, isOutput=False)
    # SBUF-native layout [p, b, t, d] (row n = 8p + t); host rearranges
    out_ext = nc.declare_dram_parameter("out", [128, BPC, NT, 3], F32, isOutput=True)

    with tile.TileContext(nc) as tc:
        with (
            tc.tile_pool(name="const", bufs=1) as constp,
            tc.tile_pool(name="kv32", bufs=1) as kv32p,
            tc.tile_pool(name="kv16", bufs=1) as kv16p,
            tc.tile_pool(name="kvT", bufs=2) as kvTp,
            tc.tile_pool(name="aug", bufs=2) as augp,
            tc.tile_pool(name="ninv", bufs=8) as ninvp,
            tc.tile_pool(name="wt", bufs=16) as wtp,
            tc.tile_pool(name="epi", bufs=2) as epip,
            tc.tile_pool(name="psd2", bufs=2, space="PSUM") as psd2,
            tc.tile_pool(name="psrel", bufs=4, space="PSUM") as psrel,
        ):
            # ---- kv loads FIRST across three DMA queues ----
            kv32s, kv16s, posf = {}, {}, {}

            def kvsrc(b, h):
                src = kv_ext[b].rearrange("(p t) c -> p t c", p=128)
                return src[:, (NT // 2) * h : (NT // 2) * (h + 1), 0 : 2 * E]

            kv16s[0] = [
                kv16p.tile([128, NT // 2, 2 * E], F16, tag=f"kv0{h}",
                           name=f"kv16_0{h}")
                for h in range(2)
            ]
            kv16s[1] = [
                kv16p.tile([128, NT // 2, 2 * E], F16, tag=f"kv1{h}",
                           name=f"kv16_1{h}")
                for h in range(2)
            ]
            kv32_00 = kv32p.tile([128, NT // 2, 2 * E], F32, tag="kv32a")
            kv32_11 = kv32p.tile([128, NT // 2, 2 * E], F32, tag="kv32b")
            nc.sync.dma_start(out=kv32_00[:, :, :], in_=kvsrc(0, 0))
            for b in range(BPC):
                posf[b] = augp.tile([128, NT, 3], F32, tag="posf", name=f"posf{b}")
                nc.scalar.dma_start(
                    out=posf[b][:, :, :],
                    in_=pos_ext[b].rearrange("(p t) d -> p t d", p=128),
                )
            nc.gpsimd.dma_start(out=kv16s[0][1][:, :, :], in_=kvsrc(0, 1))
            nc.sync.dma_start(out=kv32_11[:, :, :], in_=kvsrc(1, 1))
            nc.gpsimd.dma_start(out=kv16s[1][0][:, :, :], in_=kvsrc(1, 0))

            # ---- PE warm-up primer ----
            warm_in = constp.tile([128, 512], F16)
            nc.vector.memset(warm_in[:, :], 0.0)
            warm_ps = psrel.tile([128, 512], F32, tag="rel")
            for i in range(14):
                nc.tensor.matmul(
                    warm_ps[:, :],
                    lhsT=warm_in[:, 0:128],
                    rhs=warm_in[:, :],
                    start=(i == 0),
                    stop=(i == 13),
                )
            warm_sink = constp.tile([128, 1], F32)
            nc.vector.tensor_copy(warm_sink[:, :], warm_ps[:, 0:1])

            identity16 = constp.tile([128, 128], F16)
            make_identity(nc, identity16[:, :])
            identity32 = constp.tile([128, 128], F32)
            make_identity(nc, identity32[:, :])

            pre_all = constp.tile([128, BPC, NT, 3], F32)

            kmuT, vmuT, augTL, augTR, X, pos16f = {}, {}, {}, {}, {}, {}
            wts = {b: [] for b in range(BPC)}

            # engine casts for the raw-fp32 kv halves
            nc.vector.tensor_copy(kv16s[0][0][:, :, :], kv32_00[:, :, :])
            nc.scalar.copy(kv16s[1][1][:, :, :], kv32_11[:, :, :])

            def emit_aug(b):
                pf = posf[b]
                pos16 = augp.tile([128, NT, 3], F16, tag="p16")
                nc.gpsimd.tensor_copy(pos16[:, :, :], pf[:, :, :])
                pos16f[b] = augp.tile(
                    [128, NT, 3], F32, tag="p16f", name=f"p16f{b}"
                )
                nc.gpsimd.tensor_copy(pos16f[b][:, :, :], pos16[:, :, :])
                sq3 = augp.tile([128, NT, 3], F32, tag="sq3")
                nc.gpsimd.tensor_mul(
                    sq3[:, :, :], pos16f[b][:, :, :], pos16f[b][:, :, :]
                )
                pn2 = augp.tile([128, NT, 1], F32, tag="pn2")
                nc.vector.tensor_reduce(
                    out=pn2[:, :, :],
                    in_=sq3[:, :, :],
                    op=mybir.AluOpType.add,
                    axis=mybir.AxisListType.X,
                )
                h16 = augp.tile([128, NT, 1], F16, tag="h16")
                nc.gpsimd.tensor_copy(h16[:, :, :], pn2[:, :, :])
                h32 = augp.tile([128, NT, 1], F32, tag="h32")
                nc.gpsimd.tensor_copy(h32[:, :, :], h16[:, :, :])
                l32 = augp.tile([128, NT, 1], F32, tag="l32")
                nc.gpsimd.tensor_sub(l32[:, :, :], pn2[:, :, :], h32[:, :, :])

                # [L(7)|R(7)] in n-major; L=[p,1,1,h,l], R=[-2p,h,l,1,1]
                augb = augp.tile([128, NT, 2 * KA], F16, tag="augb")
                nc.gpsimd.tensor_copy(augb[:, :, 0:3], pos16[:, :, :])
                nc.gpsimd.memset(augb[:, :, 3:5], 1.0)
                nc.gpsimd.tensor_copy(augb[:, :, 5:6], h16[:, :, :])
                nc.gpsimd.tensor_copy(augb[:, :, 6:7], l32[:, :, :])
                nc.gpsimd.tensor_scalar_mul(augb[:, :, 7:10], pos16[:, :, :], -2.0)
                nc.gpsimd.tensor_copy(augb[:, :, 10:11], h16[:, :, :])
                nc.gpsimd.tensor_copy(augb[:, :, 11:12], l32[:, :, :])
                nc.gpsimd.memset(augb[:, :, 12:14], 1.0)

                aT_ps = psd2.tile([2 * KA, N], F16, tag="d2")
                for t in range(NT):
                    nc.tensor.transpose(
                        aT_ps[:, 128 * t : 128 * (t + 1)],
                        augb[:, t, :],
                        identity16[:, :],
                    )
                augTL[b] = augp.tile(
                    [2 * KA, N], F16, tag="augTL", name=f"augTL{b}"
                )
                # b1's drains go to scalar: the vector queue must stay free of
                # kv/pos-b1-dependent work so capmuls aren't blocked behind it
                if b == 0:
                    nc.vector.tensor_copy(augTL[b][:, :], aT_ps[:, :])
                else:
                    nc.scalar.copy(augTL[b][:, :], aT_ps[:, :])
                # row-swapped [R;L] via SBUF->SBUF DMA (engines are
                # partition-locked; DMA is not)
                augTR[b] = augp.tile(
                    [2 * KA, N], F16, tag="augTR", name=f"augTR{b}"
                )
                nc.sync.dma_start(
                    out=augTR[b][0:KA, :], in_=augTL[b][KA : 2 * KA, :]
                )
                nc.sync.dma_start(
                    out=augTR[b][KA : 2 * KA, :], in_=augTL[b][0:KA, :]
                )

                X[b] = augp.tile([128, NT, 4], F16, tag="X", name=f"X{b}")
                nc.gpsimd.tensor_copy(X[b][:, :, 0:3], pos16[:, :, :])
                nc.gpsimd.memset(X[b][:, :, 3:4], 1.0)

            def emit_kvT(b):
                kT_ps = psd2.tile([128, N], F16, tag="d2")
                for t in range(NT):
                    kv16 = kv16s[b][t // (NT // 2)]
                    tt = t % (NT // 2)
                    nc.tensor.transpose(
                        kT_ps[:, 128 * t : 128 * (t + 1)],
                        kv16[:, tt, 0:E],
                        identity16[:, :],
                    )
                kmuT[b] = kvTp.tile([128, N], F16, tag="kmuT", name=f"kmuT{b}")
                if b == 0:
                    nc.vector.tensor_copy(kmuT[b][:, :], kT_ps[:, :])
                else:
                    nc.scalar.copy(kmuT[b][:, :], kT_ps[:, :])
                vT_ps = psd2.tile([128, N], F16, tag="d2")
                for t in range(NT):
                    kv16 = kv16s[b][t // (NT // 2)]
                    tt = t % (NT // 2)
                    nc.tensor.transpose(
                        vT_ps[:, 128 * t : 128 * (t + 1)],
                        kv16[:, tt, E : 2 * E],
                        identity16[:, :],
                    )
                vmuT[b] = kvTp.tile([128, N], F16, tag="vmuT", name=f"vmuT{b}")
                nc.scalar.copy(vmuT[b][:, :], vT_ps[:, :])

            def emit_tile(b, t):
                d2_ps = psd2.tile([128, N], F32, tag="d2")
                for h in range(2):
                    cs = slice(512 * h, 512 * (h + 1))
                    nc.tensor.matmul(
                        d2_ps[:, cs],
                        lhsT=augTL[b][:, 128 * t : 128 * (t + 1)],
                        rhs=augTR[b][:, cs],
                        start=True,
                        stop=True,
                    )
                ninv = ninvp.tile([128, N], F16)
                nc.scalar.activation(
                    ninv[:, :], d2_ps[:, :], AF.Abs_reciprocal_sqrt,
                    scale=float(E) / 2.0,
                )
                w16 = wtp.tile([128, N], F16)
                for h in range(2):
                    cs = slice(512 * h, 512 * (h + 1))
                    rel_ps = psrel.tile([128, 512], F32, tag="rel")
                    nc.tensor.matmul(
                        rel_ps[:, :],
                        lhsT=vmuT[b][:, 128 * t : 128 * (t + 1)],
                        rhs=kmuT[b][:, cs],
                        start=True,
                        stop=True,
                    )
                    nc.vector._custom_dve(
                        CAPMUL_GNN,
                        out=w16[:, cs],
                        in0=rel_ps[:, :],
                        in1=ninv[:, cs],
                        s0=CAP,
                        s1=0.0,
                    )
                wts[b].append(w16)

            # ---- main: interleave both batches, b0's first tiles lead ----
            emit_aug(0)
            emit_kvT(0)
            pair_order = [(0, 0), (0, 1), (0, 2), ("aug1", None), (0, 3)]
            rest0 = [(0, t) for t in range(4, NT)]
            rest1 = [("kvT1", None)] + [(1, t) for t in range(NT)]
            while rest0 or rest1:
                if rest1:
                    pair_order.append(rest1.pop(0))
                if rest0:
                    pair_order.append(rest0.pop(0))
            for entry in pair_order:
                if entry[0] == "aug1":
                    emit_aug(1)
                elif entry[0] == "kvT1":
                    emit_kvT(1)
                else:
                    emit_tile(*entry)

            # ---- deferred P accumulation bursts + epilogues ----
            for b in range(BPC):
                P_ps = psd2.tile([4, N], F32, tag="d2", name=f"P{b}")
                for t in range(NT):
                    for h in range(2):
                        cs = slice(512 * h, 512 * (h + 1))
                        nc.tensor.matmul(
                            P_ps[:, cs],
                            lhsT=X[b][:, t, :],
                            rhs=wts[b][t][:, cs],
                            start=(t == 0),
                            stop=(t == NT - 1),
                        )
                Psb = epip.tile([4, N], F32, tag="Psb")
                nc.scalar.copy(Psb[:, :], P_ps[:, :])
                PT_ps = psrel.tile([128, NT * 4], F32, tag="rel")
                for c in range(NT):
                    nc.tensor.transpose(
                        PT_ps[:, 4 * c : 4 * (c + 1)],
                        Psb[:, 128 * c : 128 * (c + 1)],
                        identity32[0:4, 0:4],
                    )
                PT = epip.tile([128, NT, 4], F32, tag="PT")
                nc.vector.tensor_copy(
                    PT[:, :, :], PT_ps[:, :].rearrange("p (t f) -> p t f", f=4)
                )
                tmp = epip.tile([128, NT, 3], F32, tag="tmp")
                a0, a1 = bass.broadcast_tensor_aps(pos16f[b][:, :, :], PT[:, :, 3:4])
                nc.gpsimd.tensor_mul(tmp[:, :, :], a0, a1)
                nc.gpsimd.tensor_sub(pre_all[:, b, :, :], tmp[:, :, :], PT[:, :, 0:3])

            act = constp.tile([128, BPC, NT, 3], F32)
            nc.scalar.activation(act[:, :, :, :], pre_all[:, :, :, :], AF.Tanh)
            actf = constp.tile([128, BPC, NT, 3], F32)
            nc.gpsimd.tensor_scalar_mul(actf[:, :, :, :], act[:, :, :, :], ACTION_SCALE)
            nc.sync.dma_start(out=out_ext[:, :, :, :], in_=actf[:, :, :, :])

    nc.compile()
    return nc


_NC_CACHE = {}


def _get_nc():
    if "nc" not in _NC_CACHE:
        _NC_CACHE["nc"] = build_nc()
    return _NC_CACHE["nc"]


def kernel(**inputs):
    kv = np.ascontiguousarray(np.asarray(inputs["kv"], dtype=np.float32))
    pos = np.ascontiguousarray(np.asarray(inputs["positions"], dtype=np.float32))
    assert kv.shape == (B, N, CKV) and pos.shape == (B, N, 3)
    nc = _get_nc()
    in_maps = [
        {
            "kv": kv[i * BPC : (i + 1) * BPC],
            "positions": pos[i * BPC : (i + 1) * BPC],
        }
        for i in range(NCORES)
    ]
    last_err = None
    for attempt in range(3):
        try:
            res = run_bass_kernel_spmd(nc, in_maps, core_ids=list(range(NCORES)))
            break
        except Exception as e:  # transient NRT device-state races between procs
            last_err = e
            if attempt == 2:
                raise
            time.sleep(2.0 * (attempt + 1))
    outs = res.results
    # out dump is [p, b, t, d] with row n = 8p + t -> [b, n, d]
    full = [
        np.ascontiguousarray(
            outs[i]["out"].transpose(1, 0, 2, 3).reshape(BPC, N, 3)
        )
        for i in range(NCORES)
    ]
    return np.concatenate(full, axis=0)


if __name__ == "__main__":
    rng = np.random.default_rng(0)
    kv = rng.standard_normal((B, N, CKV), dtype=np.float32)
    pos = rng.standard_normal((B, N, 3), dtype=np.float32)
    out = kernel(kv=kv, positions=pos)
    print("out", out.shape, out.dtype, float(np.abs(out).max()))


# revision 58
# speedup vs baseline: 1.1423x; 1.1346x over previous
"""Trainium2 Bass kernel for nn_Actor (gnn_message_passing).

Math (per batch b):
  k_mu = kv[..., :128], v_mu = kv[..., 128:256]
  rel[n,m]  = <k_mu[n], v_mu[m]> / sqrt(128)
  P[n,m,:]  = pos[n] - pos[m];  Pn = P / (||P|| + eps)
  out[n,:]  = 0.01 * tanh( sum_m Pn[n,m,:] * rel[n,m] )

Factored form (avoids materializing [N,N,3]):
  W[n,m]   = rel[n,m] / ||P[n,m]||
  out[n,d] = 0.01 * tanh( pos16[n,d] * s[n] - (W @ [pos16|1])[n,d] ),
  s[n] = sum_m W[n,m].  The diagonal W[n,n] cancels exactly between the
  two terms because the same fp16 positions are used on both sides.

On-device pipeline per core (2 batches, data-parallel over B=16 / 8 cores):
  - kv loaded over three DMA queues (2 raw-fp32 halves + 1 gpsimd
    cast-DMA half); engine copies cast the fp32 halves to fp16
  - d2^T[m,n] = |p_n - p_m|^2 via one K=14 fp16 matmul: operands
    [p(3),1,1,h,l] x [-2p(3),h,l,1,1] with |p|^2 split fp16 hi+lo,
    both stacked in one [L|R] buffer; a single PE transpose pass plus an
    SBUF->SBUF DMA row-swap builds lhsT and rhs (result = 2*d2)
  - ScalarE ACT Abs_reciprocal_sqrt(E/2 * d2) -> ninv fp16 (folds the
    1/sqrt(E); |.| launders tiny-negative d2 from PSUM rounding)
  - VectorE custom op: W = rel * min(ninv, CAP) fp16 (CAP kills the
    rsqrt(~0)=inf on the diagonal)
  - deferred P[4,N] += [pos16|1]^T @ W^T burst per batch, epilogue
    transposes P to n-major, pre = pos16*s - A, single deferred tanh
  - output dumped in SBUF-native layout (contiguous DMA); host reorders
"""

import time

import numpy as np

import concourse.bass as bass
import concourse.bacc as bacc
import concourse.mybir as mybir
import concourse.tile as tile
import concourse.dve_ops as dve_ops
from concourse.bass_utils import run_bass_kernel_spmd
from concourse.dve_spec import Spec, minn
from concourse.dve_uop import DveOpSpec
from concourse.masks import make_identity

F32 = mybir.dt.float32
F16 = mybir.dt.float16
AF = mybir.ActivationFunctionType

B, N, CKV = 16, 1024, 259
E = 128
NCORES = 8
BPC = B // NCORES          # batches per core
NT = N // 128              # 128-row m-tiles per batch
KA = 7                     # aug rows per side (K=2*KA matmul computes 2*d2)
ACTION_SCALE = 0.01
CAP = float(np.float16(1.0 / np.sqrt(E * 5e-6)))   # ninv cap (d2 floor 5e-6)


def _register_capmul():
    name = "CAPMUL_GNN"
    if name in dve_ops._SUB_OPCODE_FOR_NAME:
        return next(op for op in dve_ops.OPS if op.name == name)
    from concourse.dve_spec import Src0, Src1, C0, lower

    body = Src0 * minn(Src1, C0)

    def _ref(in0, in1, s0, s1, imm2):
        in0 = np.asarray(in0, np.float32)
        in1 = np.asarray(in1, np.float32)
        return (in0 * np.minimum(in1, np.float32(s0))).astype(np.float32)

    spec = Spec(body=body, reference=_ref)
    opcode = dve_ops._CUSTOM_DVE_ROW_BASE + len(dve_ops.OPS)
    shas = {}
    for ver in ("v3", "v4"):
        try:
            uops = lower(spec, ver=ver)
            shas[ver] = DveOpSpec(
                name=name, opcode=opcode, uops=uops, rd1_en=True
            ).sha(ver)
        except Exception:
            pass
    op = dve_ops.DveOp(name, spec, subdim=False, uops_sha=shas)
    dve_ops.OPS.append(op)
    dve_ops.CUSTOM_DVE_SPECS[name] = spec
    dve_ops._SUB_OPCODE_FOR_NAME[name] = opcode
    return op


CAPMUL_GNN = _register_capmul()


def build_nc():
    nc = bacc.Bacc("TRN2", target_bir_lowering=False, debug=False)
    kv_ext = nc.declare_dram_parameter("kv", [BPC, N, CKV], F32, isOutput=False)
    pos_ext = nc.declare_dram_parameter("positions", [BPC, N, 3], F32, isOutput=False)
    # SBUF-native layout [p, b, t, d] (row n = 8p + t); host rearranges
    out_ext = nc.declare_dram_parameter("out", [128, BPC, NT, 3], F32, isOutput=True)

    with tile.TileContext(nc) as tc:
        with (
            tc.tile_pool(name="const", bufs=1) as constp,
            tc.tile_pool(name="kv32", bufs=1) as kv32p,
            tc.tile_pool(name="kv16", bufs=1) as kv16p,
            tc.tile_pool(name="kvT", bufs=2) as kvTp,
            tc.tile_pool(name="aug", bufs=2) as augp,
            tc.tile_pool(name="ninv", bufs=12) as ninvp,
            tc.tile_pool(name="wt", bufs=16) as wtp,
            tc.tile_pool(name="epi", bufs=2) as epip,
            tc.tile_pool(name="psd2", bufs=2, space="PSUM") as psd2,
            tc.tile_pool(name="psrel", bufs=4, space="PSUM") as psrel,
        ):
            # ---- kv loads FIRST across three DMA queues ----
            kv32s, kv16s, posf = {}, {}, {}

            def kvsrc(b, h):
                src = kv_ext[b].rearrange("(p t) c -> p t c", p=128)
                return src[:, (NT // 2) * h : (NT // 2) * (h + 1), 0 : 2 * E]

            kv16s[0] = [
                kv16p.tile([128, NT // 2, 2 * E], F16, tag=f"kv0{h}",
                           name=f"kv16_0{h}")
                for h in range(2)
            ]
            kv16s[1] = [
                kv16p.tile([128, NT // 2, 2 * E], F16, tag=f"kv1{h}",
                           name=f"kv16_1{h}")
                for h in range(2)
            ]
            kv32_00 = kv32p.tile([128, NT // 2, 2 * E], F32, tag="kv32a")
            kv32_11 = kv32p.tile([128, NT // 2, 2 * E], F32, tag="kv32b")
            nc.sync.dma_start(out=kv32_00[:, :, :], in_=kvsrc(0, 0))
            for b in range(BPC):
                posf[b] = augp.tile([128, NT, 3], F32, tag="posf", name=f"posf{b}")
                nc.scalar.dma_start(
                    out=posf[b][:, :, :],
                    in_=pos_ext[b].rearrange("(p t) d -> p t d", p=128),
                )
            nc.gpsimd.dma_start(out=kv16s[0][1][:, :, :], in_=kvsrc(0, 1))
            nc.sync.dma_start(out=kv32_11[:, :, :], in_=kvsrc(1, 1))
            nc.gpsimd.dma_start(out=kv16s[1][0][:, :, :], in_=kvsrc(1, 0))

            # ---- PE warm-up primer ----
            warm_in = constp.tile([128, 512], F16)
            nc.vector.memset(warm_in[:, :], 0.0)
            warm_ps = psrel.tile([128, 512], F32, tag="rel")
            for i in range(14):
                nc.tensor.matmul(
                    warm_ps[:, :],
                    lhsT=warm_in[:, 0:128],
                    rhs=warm_in[:, :],
                    start=(i == 0),
                    stop=(i == 13),
                )
            warm_sink = constp.tile([128, 1], F32)
            nc.vector.tensor_copy(warm_sink[:, :], warm_ps[:, 0:1])

            identity16 = constp.tile([128, 128], F16)
            make_identity(nc, identity16[:, :])
            identity32 = constp.tile([128, 128], F32)
            make_identity(nc, identity32[:, :])

            pre_all = constp.tile([128, BPC, NT, 3], F32)

            kmuT, vmuT, augTL, augTR, X, pos16f = {}, {}, {}, {}, {}, {}
            wts = {b: [] for b in range(BPC)}

            # engine casts for the raw-fp32 kv halves
            nc.vector.tensor_copy(kv16s[0][0][:, :, :], kv32_00[:, :, :])
            nc.scalar.copy(kv16s[1][1][:, :, :], kv32_11[:, :, :])

            def emit_aug(b):
                pf = posf[b]
                pos16 = augp.tile([128, NT, 3], F16, tag="p16")
                nc.gpsimd.tensor_copy(pos16[:, :, :], pf[:, :, :])
                pos16f[b] = augp.tile(
                    [128, NT, 3], F32, tag="p16f", name=f"p16f{b}"
                )
                nc.gpsimd.tensor_copy(pos16f[b][:, :, :], pos16[:, :, :])
                sq3 = augp.tile([128, NT, 3], F32, tag="sq3")
                nc.gpsimd.tensor_mul(
                    sq3[:, :, :], pos16f[b][:, :, :], pos16f[b][:, :, :]
                )
                pn2 = augp.tile([128, NT, 1], F32, tag="pn2")
                nc.vector.tensor_reduce(
                    out=pn2[:, :, :],
                    in_=sq3[:, :, :],
                    op=mybir.AluOpType.add,
                    axis=mybir.AxisListType.X,
                )
                h16 = augp.tile([128, NT, 1], F16, tag="h16")
                nc.gpsimd.tensor_copy(h16[:, :, :], pn2[:, :, :])
                h32 = augp.tile([128, NT, 1], F32, tag="h32")
                nc.gpsimd.tensor_copy(h32[:, :, :], h16[:, :, :])
                l32 = augp.tile([128, NT, 1], F32, tag="l32")
                nc.gpsimd.tensor_sub(l32[:, :, :], pn2[:, :, :], h32[:, :, :])

                # [L(7)|R(7)] in n-major; L=[p,1,1,h,l], R=[-2p,h,l,1,1]
                augb = augp.tile([128, NT, 2 * KA], F16, tag="augb")
                nc.gpsimd.tensor_copy(augb[:, :, 0:3], pos16[:, :, :])
                nc.gpsimd.memset(augb[:, :, 3:5], 1.0)
                nc.gpsimd.tensor_copy(augb[:, :, 5:6], h16[:, :, :])
                nc.gpsimd.tensor_copy(augb[:, :, 6:7], l32[:, :, :])
                nc.gpsimd.tensor_scalar_mul(augb[:, :, 7:10], pos16[:, :, :], -2.0)
                nc.gpsimd.tensor_copy(augb[:, :, 10:11], h16[:, :, :])
                nc.gpsimd.tensor_copy(augb[:, :, 11:12], l32[:, :, :])
                nc.gpsimd.memset(augb[:, :, 12:14], 1.0)

                aT_ps = psd2.tile([2 * KA, N], F16, tag="d2")
                for t in range(NT):
                    nc.tensor.transpose(
                        aT_ps[:, 128 * t : 128 * (t + 1)],
                        augb[:, t, :],
                        identity16[:, :],
                    )
                augTL[b] = augp.tile(
                    [2 * KA, N], F16, tag="augTL", name=f"augTL{b}"
                )
                # b1's drains go to scalar: the vector queue must stay free of
                # kv/pos-b1-dependent work so capmuls aren't blocked behind it
                if b == 0:
                    nc.vector.tensor_copy(augTL[b][:, :], aT_ps[:, :])
                else:
                    nc.scalar.copy(augTL[b][:, :], aT_ps[:, :])
                # row-swapped [R;L] via SBUF->SBUF DMA (engines are
                # partition-locked; DMA is not)
                augTR[b] = augp.tile(
                    [2 * KA, N], F16, tag="augTR", name=f"augTR{b}"
                )
                nc.sync.dma_start(
                    out=augTR[b][0:KA, :], in_=augTL[b][KA : 2 * KA, :]
                )
                nc.sync.dma_start(
                    out=augTR[b][KA : 2 * KA, :], in_=augTL[b][0:KA, :]
                )

                X[b] = augp.tile([128, NT, 4], F16, tag="X", name=f"X{b}")
                nc.gpsimd.tensor_copy(X[b][:, :, 0:3], pos16[:, :, :])
                nc.gpsimd.memset(X[b][:, :, 3:4], 1.0)

            def emit_kvT(b):
                kT_ps = psd2.tile([128, N], F16, tag="d2")
                for t in range(NT):
                    kv16 = kv16s[b][t // (NT // 2)]
                    tt = t % (NT // 2)
                    nc.tensor.transpose(
                        kT_ps[:, 128 * t : 128 * (t + 1)],
                        kv16[:, tt, 0:E],
                        identity16[:, :],
                    )
                kmuT[b] = kvTp.tile([128, N], F16, tag="kmuT", name=f"kmuT{b}")
                if b == 0:
                    nc.vector.tensor_copy(kmuT[b][:, :], kT_ps[:, :])
                else:
                    nc.scalar.copy(kmuT[b][:, :], kT_ps[:, :])
                vT_ps = psd2.tile([128, N], F16, tag="d2")
                for t in range(NT):
                    kv16 = kv16s[b][t // (NT // 2)]
                    tt = t % (NT // 2)
                    nc.tensor.transpose(
                        vT_ps[:, 128 * t : 128 * (t + 1)],
                        kv16[:, tt, E : 2 * E],
                        identity16[:, :],
                    )
                vmuT[b] = kvTp.tile([128, N], F16, tag="vmuT", name=f"vmuT{b}")
                nc.scalar.copy(vmuT[b][:, :], vT_ps[:, :])

            ninv_of = {}

            def emit_d2(b, t):
                # d2 + rsqrt need only positions -> b1's can run way before
                # b1's kv lands, filling the b0-solo phase
                d2_ps = psd2.tile([128, N], F32, tag="d2")
                for h in range(2):
                    cs = slice(512 * h, 512 * (h + 1))
                    nc.tensor.matmul(
                        d2_ps[:, cs],
                        lhsT=augTL[b][:, 128 * t : 128 * (t + 1)],
                        rhs=augTR[b][:, cs],
                        start=True,
                        stop=True,
                    )
                ninv = ninvp.tile([128, N], F16)
                nc.scalar.activation(
                    ninv[:, :], d2_ps[:, :], AF.Abs_reciprocal_sqrt,
                    scale=float(E) / 2.0,
                )
                ninv_of[(b, t)] = ninv

            def emit_w(b, t):
                ninv = ninv_of[(b, t)]
                w16 = wtp.tile([128, N], F16)
                for h in range(2):
                    cs = slice(512 * h, 512 * (h + 1))
                    rel_ps = psrel.tile([128, 512], F32, tag="rel")
                    nc.tensor.matmul(
                        rel_ps[:, :],
                        lhsT=vmuT[b][:, 128 * t : 128 * (t + 1)],
                        rhs=kmuT[b][:, cs],
                        start=True,
                        stop=True,
                    )
                    nc.vector._custom_dve(
                        CAPMUL_GNN,
                        out=w16[:, cs],
                        in0=rel_ps[:, :],
                        in1=ninv[:, cs],
                        s0=CAP,
                        s1=0.0,
                    )
                wts[b].append(w16)

            # ---- main: d2/rsqrt decoupled from rel/capmul; b1's d2 work
            # fills the b0-solo phase, the tail is rel/capmul-only ----
            emit_aug(0)
            emit_kvT(0)
            order = [
                ("d2", 0, 0), ("w", 0, 0), ("d2", 0, 1), ("w", 0, 1),
                ("aug1",),
                ("d2", 1, 0), ("d2", 0, 2), ("w", 0, 2),
                ("d2", 1, 1), ("d2", 0, 3), ("w", 0, 3),
                ("kvT1",),
                ("d2", 1, 2), ("w", 1, 0), ("d2", 0, 4), ("w", 0, 4),
                ("d2", 1, 3), ("w", 1, 1), ("d2", 0, 5), ("w", 0, 5),
                ("d2", 1, 4), ("w", 1, 2), ("d2", 0, 6), ("w", 0, 6),
                ("d2", 1, 5), ("w", 1, 3), ("d2", 0, 7), ("w", 0, 7),
                ("d2", 1, 6), ("w", 1, 4), ("d2", 1, 7), ("w", 1, 5),
                ("w", 1, 6), ("w", 1, 7),
            ]
            for entry in order:
                if entry[0] == "aug1":
                    emit_aug(1)
                elif entry[0] == "kvT1":
                    emit_kvT(1)
                elif entry[0] == "d2":
                    emit_d2(entry[1], entry[2])
                else:
                    emit_w(entry[1], entry[2])

            # ---- deferred P accumulation bursts + epilogues ----
            for b in range(BPC):
                P_ps = psd2.tile([4, N], F32, tag="d2", name=f"P{b}")
                for t in range(NT):
                    for h in range(2):
                        cs = slice(512 * h, 512 * (h + 1))
                        nc.tensor.matmul(
                            P_ps[:, cs],
                            lhsT=X[b][:, t, :],
                            rhs=wts[b][t][:, cs],
                            start=(t == 0),
                            stop=(t == NT - 1),
                        )
                Psb = epip.tile([4, N], F32, tag="Psb")
                nc.scalar.copy(Psb[:, :], P_ps[:, :])
                PT_ps = psrel.tile([128, NT * 4], F32, tag="rel")
                for c in range(NT):
                    nc.tensor.transpose(
                        PT_ps[:, 4 * c : 4 * (c + 1)],
                        Psb[:, 128 * c : 128 * (c + 1)],
                        identity32[0:4, 0:4],
                    )
                PT = epip.tile([128, NT, 4], F32, tag="PT")
                nc.vector.tensor_copy(
                    PT[:, :, :], PT_ps[:, :].rearrange("p (t f) -> p t f", f=4)
                )
                tmp = epip.tile([128, NT, 3], F32, tag="tmp")
                a0, a1 = bass.broadcast_tensor_aps(pos16f[b][:, :, :], PT[:, :, 3:4])
                nc.gpsimd.tensor_mul(tmp[:, :, :], a0, a1)
                nc.gpsimd.tensor_sub(pre_all[:, b, :, :], tmp[:, :, :], PT[:, :, 0:3])

            act = constp.tile([128, BPC, NT, 3], F32)
            nc.scalar.activation(act[:, :, :, :], pre_all[:, :, :, :], AF.Tanh)
            actf = constp.tile([128, BPC, NT, 3], F32)
            nc.gpsimd.tensor_scalar_mul(actf[:, :, :, :], act[:, :, :, :], ACTION_SCALE)
            nc.sync.dma_start(out=out_ext[:, :, :, :], in_=actf[:, :, :, :])

    nc.compile()
    return nc


_NC_CACHE = {}


def _get_nc():
    if "nc" not in _NC_CACHE:
        _NC_CACHE["nc"] = build_nc()
    return _NC_CACHE["nc"]


def kernel(**inputs):
    kv = np.ascontiguousarray(np.asarray(inputs["kv"], dtype=np.float32))
    pos = np.ascontiguousarray(np.asarray(inputs["positions"], dtype=np.float32))
    assert kv.shape == (B, N, CKV) and pos.shape == (B, N, 3)
    nc = _get_nc()
    in_maps = [
        {
            "kv": kv[i * BPC : (i + 1) * BPC],
            "positions": pos[i * BPC : (i + 1) * BPC],
        }
        for i in range(NCORES)
    ]
    last_err = None
    for attempt in range(3):
        try:
            res = run_bass_kernel_spmd(nc, in_maps, core_ids=list(range(NCORES)))
            break
        except Exception as e:  # transient NRT device-state races between procs
            last_err = e
            if attempt == 2:
                raise
            time.sleep(2.0 * (attempt + 1))
    outs = res.results
    # out dump is [p, b, t, d] with row n = 8p + t -> [b, n, d]
    full = [
        np.ascontiguousarray(
            outs[i]["out"].transpose(1, 0, 2, 3).reshape(BPC, N, 3)
        )
        for i in range(NCORES)
    ]
    return np.concatenate(full, axis=0)


if __name__ == "__main__":
    rng = np.random.default_rng(0)
    kv = rng.standard_normal((B, N, CKV), dtype=np.float32)
    pos = rng.standard_normal((B, N, 3), dtype=np.float32)
    out = kernel(kv=kv, positions=pos)
    print("out", out.shape, out.dtype, float(np.abs(out).max()))
